# revision 15
# baseline (speedup 1.0000x reference)
"""ClassCapsule dynamic-routing kernel for 8 Trainium2 NeuronCores.

Problem (hardcoded shapes):
    x:    [64, 2048, 16]  fp32
    W:    [2048, 16, 1024] fp32
    bias: [64, 16]        fp32
    out:  [64, 64, 16]    fp32  (squeezed v after 3 routing iterations)

Strategy (batch-sharded, no collectives):
  - B=64 split across 8 cores (8 batches each).
  - u_hat = einsum('bij,ijk->bik') computed on the PE via a block-diagonal
    trick: 8 in_caps share one matmul; lhsT is a host-prepared block-diagonal
    arrangement of x with K=(i_sub,e)=128, M=(i_sub,b)=64.
  - u_hat tiles [128=(i_sub16,b8), 1024] stored to DRAM (bf16), re-read for
    the 2 remaining routing iterations.
  - Routing per tile: agreement = reduce_d(u_hat*v) (vector), softmax over
    n_caps (ACT exp + vector reciprocal), weighted sum over in_caps via a
    constant 0/1 selector matmul on the PE accumulating in PSUM.
"""

import numpy as np

import concourse.bass as bass
import concourse.tile as tile
from concourse import bacc, mybir
from concourse.bass_utils import run_bass_kernel_spmd

# ---------------------------------------------------------------- constants
B, IC, E = 64, 2048, 16          # batch, in_caps, in_dim
NCAP, D = 64, 16                 # n_caps, cap_dim
ND = NCAP * D                    # 1024
CORES = 8
BL = B // CORES                  # 8 local batches
IB8 = IC // 8                    # 256 blocks of 8 in_caps (matmul granularity)
NT = IC // 16                    # 128 u_hat tiles of 16 in_caps
EPS = 1e-7

FP = mybir.dt.float32
BF = mybir.dt.bfloat16


def _host_prep(x, W, bias):
    """Build per-core host-side tensors."""
    # Block-diagonal x for the projection matmuls.
    # lhsT[blk][(i_sub*16+e), (j_sub*8+b)] = x[b, blk*8+j_sub, e] * (i_sub==j_sub)
    # -> per core: [IB8, 128, 64] fp32
    w_r = W.reshape(IB8, 8 * E, ND)  # [256, 128, 1024]
    # wx[blk] = [128, 1024 + 64]: W block columns then block-diagonal x columns,
    # so ONE dma per block feeds both matmul operands (single sync wait on PE).
    wx_all = []
    for c in range(CORES):
        xc = x[c * BL:(c + 1) * BL]                      # [8, 2048, 16]
        wx = np.zeros((IB8, 128, ND + 8 * BL), dtype=np.float32)
        wx[:, :, :ND] = w_r
        # fill diagonal blocks: rows i_sub*16+e, cols ND + i_sub*8+b
        xr = xc.transpose(1, 2, 0).reshape(IB8, 8, E, BL)  # [blk, i_sub, e, b]
        for s in range(8):
            wx[:, s * E:(s + 1) * E, ND + s * BL:ND + (s + 1) * BL] = xr[:, s]
        wx_all.append(wx)

    # selector: sel8[p, b] = 1 if p % 8 == b   (partition p = i_sub*8 + b)
    sel8 = np.zeros((128, BL), dtype=np.float32)
    sel8[np.arange(128), np.arange(128) % BL] = 1.0

    bias_f = np.tile(bias.reshape(1, ND), (BL, 1)).astype(np.float32)  # [8, 1024]
    return wx_all, sel8, bias_f


def _build_program():
    nc = bacc.Bacc("TRN2", target_bir_lowering=False)

    wx_d = nc.dram_tensor("wx", [IB8, 128, ND + 8 * BL], FP, kind="ExternalInput")
    sel8_d = nc.dram_tensor("sel8", [128, BL], FP, kind="ExternalInput")
    bias_d = nc.dram_tensor("bias_f", [BL, ND], FP, kind="ExternalInput")
    v_out = nc.dram_tensor("v_out", [BL, ND], FP, kind="ExternalOutput")

    u_hat_d = nc.dram_tensor("u_hat_d", [NT, 128, ND], BF)   # internal scratch
    v_scr = nc.dram_tensor("v_scr", [BL, ND], BF)            # bcast bounce

    with tile.TileContext(nc) as tc:
        with (
            tc.tile_pool(name="wp", bufs=4) as wp,
            tc.tile_pool(name="up", bufs=3) as up,
            tc.tile_pool(name="tp", bufs=3) as tp,
            tc.tile_pool(name="smalls", bufs=4) as sp,
            tc.tile_pool(name="consts", bufs=1) as cp,
            tc.tile_pool(name="vb", bufs=2) as vbp,
            tc.tile_pool(name="ps", bufs=2, space="PSUM") as psp,
            tc.tile_pool(name="ps_acc", bufs=1, space="PSUM") as psa,
            tc.tile_pool(name="bstate", bufs=1) as bsp,
        ):
            # ---- constants resident in SBUF
            sel8_f = cp.tile([128, BL], FP)
            nc.sync.dma_start(out=sel8_f, in_=sel8_d[:, :])
            sel8_b = cp.tile([128, BL], BF)
            nc.scalar.copy(out=sel8_b, in_=sel8_f)
            bias_sb = cp.tile([BL, ND], FP)
            nc.sync.dma_start(out=bias_sb, in_=bias_d[:, :])
            eps_t = cp.tile([BL, 1], FP)
            nc.vector.memset(eps_t, EPS)

            # routing logits state: [128, NT*64]
            b_all = bsp.tile([128, NT * NCAP], FP)

            # ---------------- squash helper: v = squash(s_psum*scale + bias)
            def squash_from_psum(s_ps, scale):
                s_sb = sp.tile([BL, ND], FP, tag="s_sb")
                # s = s_ps*scale + bias
                nc.vector.scalar_tensor_tensor(
                    out=s_sb, in0=s_ps, scalar=float(scale), in1=bias_sb,
                    op0=mybir.AluOpType.mult, op1=mybir.AluOpType.add)
                sq = sp.tile([BL, ND], FP, tag="sq")
                nc.vector.tensor_mul(sq, s_sb, s_sb)
                nsq = sp.tile([BL, NCAP], FP, tag="nsq")
                nc.vector.reduce_sum(
                    out=nsq, in_=sq.rearrange("p (n d) -> p n d", d=D),
                    axis=mybir.AxisListType.X)
                norm = sp.tile([BL, NCAP], FP, tag="norm")
                # norm = sqrt(nsq + EPS)
                nc.scalar.activation(out=norm, in_=nsq,
                                     func=mybir.ActivationFunctionType.Sqrt,
                                     bias=eps_t[:, :], scale=1.0)
                den = sp.tile([BL, NCAP], FP, tag="den")
                # den = (nsq + EPS + 1) * norm
                nc.vector.scalar_tensor_tensor(
                    out=den, in0=nsq, scalar=float(EPS + 1.0), in1=norm,
                    op0=mybir.AluOpType.add, op1=mybir.AluOpType.mult)
                rden = sp.tile([BL, NCAP], FP, tag="rden")
                nc.vector.reciprocal(out=rden, in_=den)
                fac = sp.tile([BL, NCAP], FP, tag="fac")
                # fac = (nsq + EPS) * rden
                nc.vector.scalar_tensor_tensor(
                    out=fac, in0=nsq, scalar=float(EPS), in1=rden,
                    op0=mybir.AluOpType.add, op1=mybir.AluOpType.mult)
                v_sb = sp.tile([BL, ND], FP, tag="v_sb")
                fac_b = bass.AP(tensor=fac.tensor, offset=fac.offset,
                                ap=[list(fac.ap[0]), list(fac.ap[1]), [0, D]])
                nc.vector.tensor_mul(
                    v_sb.rearrange("p (n d) -> p n d", d=D),
                    s_sb.rearrange("p (n d) -> p n d", d=D),
                    fac_b)
                return s_sb, v_sb

            def broadcast_v(v_sb):
                """v_sb [8, 1024] fp32 -> vb [128, 1024] bf16 (partition bcast)."""
                v_bf = sp.tile([BL, ND], BF, tag="v_bf")
                nc.vector.tensor_copy(out=v_bf, in_=v_sb)
                nc.sync.dma_start(out=v_scr[:, :], in_=v_bf)
                vb = vbp.tile([128, ND], BF, tag="vb")
                src = bass.AP(tensor=v_scr, offset=0,
                              ap=[[0, 128 // BL], [ND, BL], [1, ND]])
                nc.sync.dma_start(out=vb, in_=src)
                return vb

            # ================= Phase P: projection + iter-0 sum =================
            s0_ps = psa.tile([BL, ND], FP, tag="s_acc")
            for t in range(NT):
                u_ps = psp.tile([128, ND], FP, tag="u_ps")
                for h in range(2):  # two 8-in_cap blocks -> partitions h*64..
                    blk = 2 * t + h
                    wt = wp.tile([128, ND + 8 * BL], FP, tag="w")
                    nc.sync.dma_start(out=wt, in_=wx_d[blk])
                    for nh in range(2):  # N halves of 512
                        nc.tensor.matmul(
                            u_ps[h * 64:(h + 1) * 64, nh * 512:(nh + 1) * 512],
                            wt[:, ND:ND + 8 * BL],
                            wt[:, nh * 512:(nh + 1) * 512],
                            start=True, stop=True)
                u_bf = up.tile([128, ND], BF, tag="u_bf")
                nc.scalar.copy(out=u_bf[:, 0:512], in_=u_ps[:, 0:512])
                nc.scalar.copy(out=u_bf[:, 512:1024], in_=u_ps[:, 512:1024])
                nc.sync.dma_start(out=u_hat_d[t], in_=u_bf)
                for nh in range(2):
                    nc.tensor.matmul(
                        s0_ps[:, nh * 512:(nh + 1) * 512],
                        sel8_b, u_bf[:, nh * 512:(nh + 1) * 512],
                        start=(t == 0), stop=(t == NT - 1),
                        skip_group_check=True)

            _, v_sb = squash_from_psum(s0_ps, 1.0 / NCAP)
            vb = broadcast_v(v_sb)

            # ================= Routing iterations 1 and 2 =================
            for it in (1, 2):
                s_ps = psa.tile([BL, ND], FP, tag="s_acc")
                for t in range(NT):
                    u_bf = up.tile([128, ND], BF, tag="u_bf")
                    nc.sync.dma_start(out=u_bf, in_=u_hat_d[t])
                    tmp = tp.tile([128, ND], BF, tag="tmp")
                    nc.gpsimd.tensor_mul(tmp, u_bf, vb)
                    b_slice = b_all[:, t * NCAP:(t + 1) * NCAP]
                    if it == 1:
                        # b starts at zero: agreement goes straight into b
                        nc.vector.reduce_sum(
                            out=b_slice,
                            in_=tmp.rearrange("p (n d) -> p n d", d=D),
                            axis=mybir.AxisListType.X)
                    else:
                        agr = sp.tile([128, NCAP], FP, tag="agr")
                        nc.vector.reduce_sum(
                            out=agr,
                            in_=tmp.rearrange("p (n d) -> p n d", d=D),
                            axis=mybir.AxisListType.X)
                        nc.vector.tensor_add(b_slice, b_slice, agr)
                    c_un = sp.tile([128, NCAP], FP, tag="c_un")
                    se = sp.tile([128, 1], FP, tag="se")
                    nc.scalar.activation(out=c_un, in_=b_slice,
                                         func=mybir.ActivationFunctionType.Exp,
                                         accum_out=se)
                    rec = sp.tile([128, 1], FP, tag="rec")
                    nc.vector.reciprocal(out=rec, in_=se)
                    c_bf = sp.tile([128, NCAP], BF, tag="c_bf")
                    nc.scalar.mul(c_bf, c_un, rec)
                    w_bf = tp.tile([128, ND], BF, tag="w_bf")
                    c_b = bass.AP(tensor=c_bf.tensor, offset=c_bf.offset,
                                  ap=[list(c_bf.ap[0]), list(c_bf.ap[1]), [0, D]])
                    nc.vector.tensor_mul(
                        w_bf.rearrange("p (n d) -> p n d", d=D),
                        u_bf.rearrange("p (n d) -> p n d", d=D),
                        c_b)
                    for nh in range(2):
                        nc.tensor.matmul(
                            s_ps[:, nh * 512:(nh + 1) * 512],
                            sel8_b, w_bf[:, nh * 512:(nh + 1) * 512],
                            start=(t == 0), stop=(t == NT - 1),
                            skip_group_check=True)
                _, v_sb = squash_from_psum(s_ps, 1.0)
                if it < 2:
                    vb = broadcast_v(v_sb)
                else:
                    nc.sync.dma_start(out=v_out[:, :], in_=v_sb)

    nc.compile()
    return nc


_CACHED = {}


def _get_program():
    if "nc" not in _CACHED:
        _CACHED["nc"] = _build_program()
    return _CACHED["nc"]


def kernel(x, W, bias):
    x = np.asarray(x, dtype=np.float32)
    W = np.asarray(W, dtype=np.float32)
    bias = np.asarray(bias, dtype=np.float32)

    wx_all, sel8, bias_f = _host_prep(x, W, bias)
    nc = _get_program()

    in_maps = []
    for c in range(CORES):
        in_maps.append({
            "wx": wx_all[c],
            "sel8": sel8,
            "bias_f": bias_f,
        })
    res = run_bass_kernel_spmd(nc, in_maps, core_ids=list(range(CORES)))
    _CACHED["last_results"] = res
    outs = [res.results[c]["v_out"].reshape(BL, NCAP, D) for c in range(CORES)]
    return np.concatenate(outs, axis=0)
